# revision 1
# baseline (speedup 1.0000x reference)
"""Trainium2 Bass kernel for nn_AttentionModel_88905823027207.

Full inputs:  x [4, 2048, 1024] f32, w_qkv [1024, 3072] f32, w_out [1024, 1024] f32
Full output:  [4, 2048, 1024] f32  (multi-head attention, 16 heads, + out proj)

Sharding: 8 cores = (batch b in 0..3) x (head-group g in 0..1).
Each core computes 8 heads of one batch element and the partial out-projection
for its head-group's rows of w_out; the host sums the two partials per batch.

Per-core kernel (all matmuls bf16 with fp32 PSUM accumulation):
  stage1: qT,kT [512, S] = w{q,k}.T @ x.T ; vhat [S, 8*65] = x @ wv with a
          ones-column appended per head (so attnV also accumulates the
          softmax denominator as output row 64).
  attention (per head-pair, row-packed across PE row-groups via partition
          offsets 0/64): scoresT [k, q] -> ACT exp(0.125*s) PSUM->SBUF bf16
          -> attnV accumulation [65, 512] in PSUM.
  normalize: DVE reciprocal of row 64, GPSIMD partition-broadcast, DVE mul.
  out-proj: attn_normT @ w_out rows, fp32 partial out to DRAM.
"""

import numpy as np
import ml_dtypes

BF16 = ml_dtypes.bfloat16

# Full-problem dims (hardcoded per harness contract)
B_FULL, S_FULL, D_FULL, H_FULL, HD = 4, 2048, 1024, 16, 64
N_CORES = 8
HEADS_PER_CORE = H_FULL // 2  # 8


def build_nc(S=2048, D=1024, heads=8, debug=False, stage='full'):
    """Build + compile the per-core Bass program. Dims parameterizable for
    small-scale simulation; defaults are the real shapes."""
    import concourse.bass as bass
    import concourse.mybir as mybir
    import concourse.tile as tile
    from concourse import bacc

    f32 = mybir.dt.float32
    bf16 = mybir.dt.bfloat16
    FT = mybir.ActivationFunctionType

    E = heads * HD              # per-core head channels (512)
    NDT = D // 128              # d-tiles (8)
    NST = S // 128              # s-tiles / k-tiles (16)
    NSC = S // 512              # 512-wide s-chunks (4)
    NET = E // 128              # e-tiles == head pairs (4)
    NQC = S // 512              # q-chunks (4)
    assert NQC % NET == 0 or NET % NQC == 0
    VW = 65                     # v columns per head incl. ones column

    nc = bacc.Bacc("TRN2", target_bir_lowering=False, debug=debug)

    xT_d = nc.dram_tensor("xT", [D, S], bf16, kind="ExternalInput")
    wq_d = nc.dram_tensor("wq", [D, E], bf16, kind="ExternalInput")
    wk_d = nc.dram_tensor("wk", [D, E], bf16, kind="ExternalInput")
    wv_d = nc.dram_tensor("wv", [D, E], bf16, kind="ExternalInput")
    wo_d = nc.dram_tensor("wo", [E, D], bf16, kind="ExternalInput")
    out_d = nc.dram_tensor("out", [S, D], f32, kind="ExternalOutput")

    from contextlib import ExitStack

    with tile.TileContext(nc) as tc, ExitStack() as ctx:
        const = ctx.enter_context(tc.tile_pool(name="const", bufs=1))
        proj_ps = ctx.enter_context(tc.tile_pool(name="proj_ps", bufs=2, space="PSUM"))
        scores_ps = ctx.enter_context(tc.tile_pool(name="scores_ps", bufs=2, space="PSUM"))
        attn_ps = ctx.enter_context(tc.tile_pool(name="attn_ps", bufs=2, space="PSUM"))
        expp = ctx.enter_context(tc.tile_pool(name="expp", bufs=3))
        attnsb = ctx.enter_context(tc.tile_pool(name="attnsb", bufs=4))
        recipp = ctx.enter_context(tc.tile_pool(name="recipp", bufs=4))
        bcastp = ctx.enter_context(tc.tile_pool(name="bcastp", bufs=4))
        outst = ctx.enter_context(tc.tile_pool(name="outst", bufs=4))
        dramp = ctx.enter_context(tc.tile_pool(name="dramp", bufs=8, space="DRAM"))

        # ---- persistent SBUF tensors ----
        xT_sb = const.tile([128, NDT, S], bf16, tag="xT_sb")
        wq_sb = const.tile([128, NDT, E], bf16, tag="wq_sb")
        wk_sb = const.tile([128, NDT, E], bf16, tag="wk_sb")
        wv_sb = const.tile([128, NDT, E], bf16, tag="wv_sb")
        wo_sb = const.tile([128, NET, D], bf16, tag="wo_sb")
        qT = [const.tile([128, S], bf16, tag=f"qT{p}", name=f"qT{p}") for p in range(NET)]
        kT = [const.tile([128, S], bf16, tag=f"kT{p}", name=f"kT{p}") for p in range(NET)]
        vhat = [const.tile([128, heads, VW], bf16, tag=f"vh{st}", name=f"vh{st}") for st in range(NST)]
        attn_norm = [const.tile([128, S], bf16, tag=f"an{p}", name=f"an{p}") for p in range(NET)]

        # ---- input DMAs ----
        nc.sync.dma_start(out=xT_sb, in_=xT_d.ap().rearrange("(t p) s -> p t s", p=128))
        nc.sync.dma_start(out=wq_sb, in_=wq_d.ap().rearrange("(t p) e -> p t e", p=128))
        nc.sync.dma_start(out=wk_sb, in_=wk_d.ap().rearrange("(t p) e -> p t e", p=128))
        nc.sync.dma_start(out=wv_sb, in_=wv_d.ap().rearrange("(t p) e -> p t e", p=128))
        nc.sync.dma_start(out=wo_sb, in_=wo_d.ap().rearrange("(t p) d -> p t d", p=128))

        # ---- stage 1: vhat = x @ wv (+ ones columns) ----
        for st in range(NST):
            nc.vector.memset(vhat[st], 1.0)
            ps = proj_ps.tile([128, E], f32, tag="proj")
            for dt in range(NDT):
                nc.tensor.matmul(
                    ps,
                    lhsT=xT_sb[:, dt, st * 128:(st + 1) * 128],
                    rhs=wv_sb[:, dt, :],
                    start=(dt == 0),
                    stop=(dt == NDT - 1),
                )
            nc.vector.tensor_copy(
                out=vhat[st][:, :, 0:HD],
                in_=ps.rearrange("p (h c) -> p h c", c=HD),
            )

        # ---- stage 1: qT, kT = w.T @ xT ----
        for p in range(NET):
            for w_sb, dstT in ((wq_sb, qT[p]), (wk_sb, kT[p])):
                for sc in range(NSC):
                    ps = proj_ps.tile([128, 512], f32, tag="proj")
                    for dt in range(NDT):
                        nc.tensor.matmul(
                            ps,
                            lhsT=w_sb[:, dt, p * 128:(p + 1) * 128],
                            rhs=xT_sb[:, dt, sc * 512:(sc + 1) * 512],
                            start=(dt == 0),
                            stop=(dt == NDT - 1),
                        )
                    nc.vector.tensor_copy(out=dstT[:, sc * 512:(sc + 1) * 512], in_=ps)

        # ---- attention ----
        if stage == "s1":
            # debug: write stage1 vhat (v values) instead of attention output
            W = min(512, D)
            for st in range(NST):
                ot = outst.tile([128, W], f32, tag="ot")
                nh = min(W, heads * HD) // HD
                nc.vector.tensor_copy(
                    out=ot.rearrange("p (h c) -> p h c", c=HD)[:, 0:nh, :],
                    in_=vhat[st][:, 0:nh, 0:HD])
                nc.sync.dma_start(out=out_d.ap()[st * 128:(st + 1) * 128, 0:W], in_=ot)
        for qc in (range(NQC) if stage != "s1" else []):
            for p in range(NET):
                hA, hB = 2 * p, 2 * p + 1
                aA = attn_ps.tile([VW, 512], f32, tag="attnA", bufs=1, name=f"aA{qc}_{p}")
                aB = attn_ps.tile([VW, 512], f32, tag="attnB", bufs=1, name=f"aB{qc}_{p}")
                for kt in range(NST):
                    sc_ps = scores_ps.tile([128, 1024], f32, tag="scores")
                    # scoresT for the head pair, row-packed (partitions 0-63 / 64-127)
                    nc.tensor.matmul(
                        sc_ps[:, 0:512],
                        lhsT=kT[p][0:HD, kt * 128:(kt + 1) * 128],
                        rhs=qT[p][0:HD, qc * 512:(qc + 1) * 512],
                        start=True, stop=True,
                    )
                    nc.tensor.matmul(
                        sc_ps[:, 512:1024],
                        lhsT=kT[p][64:64 + HD, kt * 128:(kt + 1) * 128],
                        rhs=qT[p][64:64 + HD, qc * 512:(qc + 1) * 512],
                        start=True, stop=True,
                    )
                    ex = expp.tile([128, 1024], bf16, tag="exp")
                    nc.scalar.activation(out=ex, in_=sc_ps, func=FT.Exp, scale=0.125)
                    # attnV accumulation: one K=128 matmul per (kt, head)
                    if stage == "scores":
                        continue
                    nc.tensor.matmul(
                        aA, lhsT=vhat[kt][:, hA, :], rhs=ex[:, 0:512],
                        start=(kt == 0), stop=(kt == NST - 1), skip_group_check=True,
                    )
                    nc.tensor.matmul(
                        aB, lhsT=vhat[kt][:, hB, :], rhs=ex[:, 512:1024],
                        start=(kt == 0), stop=(kt == NST - 1), skip_group_check=True,
                    )
                # evacuate + normalize
                if stage == "scores":
                    W = min(512, D)
                    exd = outst.tile([128, W], f32, tag="ot")
                    nc.vector.tensor_copy(out=exd, in_=ex[:, 0:W])
                    if qc == 0 and p == 0:
                        nc.sync.dma_start(out=out_d.ap()[0:128, 0:W], in_=exd)
                    continue
                a_sbA = attnsb.tile([VW, 512], f32, tag="asb")
                a_sbB = attnsb.tile([VW, 512], f32, tag="asb")
                nc.vector.tensor_copy(out=a_sbA, in_=aA)
                nc.vector.tensor_copy(out=a_sbB, in_=aB)
                if stage == "nonorm":
                    nc.vector.tensor_copy(
                        out=attn_norm[p][0:64, qc * 512:(qc + 1) * 512], in_=a_sbA[0:64, :])
                    nc.vector.tensor_copy(
                        out=attn_norm[p][64:128, qc * 512:(qc + 1) * 512], in_=a_sbB[0:64, :])
                    continue
                rcA = recipp.tile([1, 512], f32, tag="rcA")
                rcB = recipp.tile([1, 512], f32, tag="rcB")
                nc.vector.reciprocal(out=rcA, in_=a_sbA[64:65, :])
                nc.vector.reciprocal(out=rcB, in_=a_sbB[64:65, :])
                bcA = bcastp.tile([64, 512], f32, tag="bcA")
                bcB = bcastp.tile([64, 512], f32, tag="bcB")
                rcd = dramp.tile([2, 512], f32, tag="rcd", name=f"rcd{qc}_{p}")
                nc.sync.dma_start(out=rcd[0:1, :], in_=rcA)
                nc.sync.dma_start(out=rcd[1:2, :], in_=rcB)
                srcA = rcd[0:1, :]
                srcB = rcd[1:2, :]
                bcastA_ap = bass.AP(tensor=srcA.tensor, offset=srcA.offset, ap=[[0, 64], [1, 512]])
                bcastB_ap = bass.AP(tensor=srcB.tensor, offset=srcB.offset, ap=[[0, 64], [1, 512]])
                nc.sync.dma_start(out=bcA[0:64, :], in_=bcastA_ap)
                nc.sync.dma_start(out=bcB[0:64, :], in_=bcastB_ap)
                nc.vector.tensor_mul(
                    attn_norm[p][0:64, qc * 512:(qc + 1) * 512],
                    a_sbA[0:64, :], bcA[0:64, :],
                )
                nmB = bcastp.tile([64, 512], bf16, tag="nmB")
                nc.vector.tensor_mul(nmB[0:64, :], a_sbB[0:64, :], bcB[0:64, :])
                nc.sync.dma_start(
                    out=attn_norm[p][64:128, qc * 512:(qc + 1) * 512], in_=nmB[0:64, :])
            # ---- out-projection for the s-tiles covered by this q-chunk ----
            DC = min(512, D)
            for st in (range(qc * (NST // NQC), (qc + 1) * (NST // NQC)) if stage != "scores" else []):
                for dc in range(D // DC):
                    ps = proj_ps.tile([128, DC], f32, tag="proj")
                    for p in range(NET):
                        nc.tensor.matmul(
                            ps,
                            lhsT=attn_norm[p][:, st * 128:(st + 1) * 128],
                            rhs=wo_sb[:, p, dc * DC:(dc + 1) * DC],
                            start=(p == 0),
                            stop=(p == NET - 1),
                        )
                    ot = outst.tile([128, DC], f32, tag="ot")
                    nc.vector.tensor_copy(out=ot, in_=ps)
                    nc.sync.dma_start(
                        out=out_d.ap()[st * 128:(st + 1) * 128, dc * DC:(dc + 1) * DC],
                        in_=ot,
                    )

    nc.compile()
    return nc


_NC_CACHE = {}


def _get_nc():
    if "nc" not in _NC_CACHE:
        _NC_CACHE["nc"] = build_nc()
    return _NC_CACHE["nc"]


def shard_inputs(x, w_qkv, w_out):
    """Host-side shard + layout prep. Returns in_maps for 8 cores."""
    D = D_FULL
    E = HEADS_PER_CORE * HD
    in_maps = []
    for core in range(N_CORES):
        b, g = core // 2, core % 2
        cs = slice(g * E, (g + 1) * E)
        in_maps.append({
            "xT": np.ascontiguousarray(x[b].T).astype(BF16),
            "wq": w_qkv[:, 0 * D:1 * D][:, cs].astype(BF16),
            "wk": w_qkv[:, 1 * D:2 * D][:, cs].astype(BF16),
            "wv": w_qkv[:, 2 * D:3 * D][:, cs].astype(BF16),
            "wo": w_out[cs, :].astype(BF16),
        })
    return in_maps


def kernel(x, w_qkv, w_out):
    from concourse.bass_utils import run_bass_kernel_spmd

    x = np.asarray(x)
    w_qkv = np.asarray(w_qkv)
    w_out = np.asarray(w_out)
    nc = _get_nc()
    in_maps = shard_inputs(x, w_qkv, w_out)
    res = run_bass_kernel_spmd(nc, in_maps, list(range(N_CORES)))
    outs = [res.results[i]["out"] for i in range(N_CORES)]
    full = np.empty((B_FULL, S_FULL, D_FULL), np.float32)
    for b in range(B_FULL):
        full[b] = outs[2 * b] + outs[2 * b + 1]
    return full

